# revision 7
# baseline (speedup 1.0000x reference)
"""AttentionalSplatting Trainium2 kernel v2 (8 NeuronCores, SPMD).

Sharding: 8 cores = T(4) x HW-halves(2).  Core c handles t = c//2 and pixel
columns [(c%2)*1152, (c%2+1)*1152).  No cross-core communication.

v2 redesign vs v1:
- spatial bias enters as host-precomputed exp(bias); attention weights are
  exp(s)*expB via DVE/Pool multiplies instead of PE identity-matmul
  injection (saves ~74k PE cycles/core).
- scores matmuls run in fp8e4m3 DoubleRow (2 k-tiles of 16 partitions),
  halving score streaming time. sqrt(SCALE) is folded into both Q and K
  during qk-norm so fp8 operands stay in normal range.
- Wo and W_out_w are fused into one matrix on the host.
- coordinate projection uses an exact hi/lo split of positions in bf16
  (integer part exact, fractional in [-0.5,0.5]) with the coord bias as a
  fifth contraction row; only the RoPE angle outer-product stays f32.
- main loop blocks pixels by 384 so each Act exp instruction covers
  (2 heads x 384) rows of one PSUM tile, reducing per-instruction Act
  access bubbles.
"""

import math
import sys

import numpy as np

sys.path.insert(0, "/opt/trn_rl_repo")

import ml_dtypes  # noqa: E402

import concourse.bass as bass  # noqa: E402
import concourse.bacc as bacc  # noqa: E402
import concourse.tile as tile  # noqa: E402
from concourse import mybir  # noqa: E402
from concourse.bass_utils import run_bass_kernel_spmd  # noqa: E402

T, M, HW, DF, H = 4, 1024, 2304, 256, 8
DKH = DF // H  # 32
QH = HW // 2  # 1152 pixels per core
SCALE = 1.0 / math.sqrt(DKH)
RSC = math.sqrt(SCALE)
D_HALF = DF // 2  # 128
D_QUART = DF // 4  # 64
THETA = (100.0 ** (-2.0 * np.arange(D_QUART, dtype=np.float32) / D_HALF)).astype(
    np.float32
)

F32 = mybir.dt.float32
BF16 = mybir.dt.bfloat16
FP8 = mybir.dt.float8e4
AF = mybir.ActivationFunctionType
DR = mybir.MatmulPerfMode.DoubleRow
BF16NP = ml_dtypes.bfloat16

N_CORES = 8
QB = 480
Q_BLOCKS = [(0, 480), (480, 480), (960, 192)]
K_BLOCKS = [(0, 384), (384, 384), (768, 256)]

USE_FP8 = True
USE_BRD = True
USE_GPD = True


def _bf(x):
    return np.ascontiguousarray(np.asarray(x, np.float32)).astype(BF16NP)


def _f32(x):
    return np.ascontiguousarray(np.asarray(x, np.float32))


def _host_constants(Wq, Wk, Wv, Wo, W_out_w, W_out_b, W_coord_w, W_coord_b):
    """Shared (core-independent) device constants, host-precomputed."""
    # pair-swapped coord weights for RoPE (swap even/odd output columns)
    perm = np.arange(DF)
    perm = perm.reshape(-1, 2)[:, ::-1].reshape(-1)
    wcsw = W_coord_w[:, perm]
    wcbsw = W_coord_b[perm]
    # coord projection lhsT with hi/lo rows + bias row:
    # rhs rows are [hi_x, lo_x, hi_y, lo_y, ones]
    wc5 = np.stack(
        [W_coord_w[0], W_coord_w[0], W_coord_w[1], W_coord_w[1], W_coord_b]
    )
    wc5s = np.stack([wcsw[0], wcsw[0], wcsw[1], wcsw[1], wcbsw])
    # signed duplicated theta: [-t0, +t0, -t1, +t1, ...], split into an
    # exact bf16 hi part + bf16 lo remainder so the angle outer product can
    # run as a bf16 matmul without precision loss; plus a pi/2 row (cos).
    ths = np.empty((D_HALF,), np.float32)
    ths[0::2] = -THETA
    ths[1::2] = THETA
    th_h = ths.astype(BF16NP).astype(np.float32)
    th_l = ths - th_h
    th = np.stack([th_h, th_h, th_l, th_l, np.full((D_HALF,), np.pi / 2.0)])
    # block-ones for per-head sum of squares: dtile k maps its 128 feature
    # rows onto head columns 4k..4k+3
    bones = np.zeros((2, 128, 8), np.float32)
    for k in range(2):
        for d in range(128):
            bones[k, d, 4 * k + d // 32] = 1.0
    # expand per-head scalars (8, q) back to the 128 feature rows of dtile k,
    # scaled by sqrt(SCALE) so fp8 Q and K each carry half the 1/sqrt(dk)
    exp8 = np.zeros((2, 8, 128), np.float32)
    for k in range(2):
        for d in range(128):
            exp8[k, 4 * k + d // 32, d] = RSC
    # selector: extract attend-psum rows 0 and 64 (the ones-column sums)
    sel = np.zeros((128, 2), np.float32)
    sel[0, 0] = 1.0
    sel[64, 1] = 1.0
    # expand the two per-pair inv-sums to paired attend-output rows
    expP2 = np.zeros((2, 128), np.float32)
    expP2[0, 1:33] = 1.0
    expP2[1, 65:97] = 1.0
    # fused output projection W2 = Wo @ W_out_w in the paired attend-output
    # row layout (sumexp rows contribute 0)
    W2 = Wo @ W_out_w
    w2aug = np.zeros((4, 128, DF), np.float32)
    for j in range(4):
        w2aug[j, 1:33, :] = W2[(2 * j) * 32 : (2 * j + 1) * 32, :]
        w2aug[j, 65:97, :] = W2[(2 * j + 1) * 32 : (2 * j + 2) * 32, :]
    return {
        "wq": _bf(Wq),
        "wk": _bf(Wk),
        "wv": _bf(Wv),
        "w2aug": _bf(w2aug),
        "woutb": _f32(W_out_b).reshape(DF, 1),
        "wc5": _bf(wc5),
        "wc5s": _bf(wc5s),
        "thc": _bf(th),
        "bones": _bf(bones),
        "exp8": _bf(exp8),
        "sel": _bf(sel),
        "expP2": _bf(expP2),
    }


_NC_CACHE = None


def _build_nc(reps=1):
    nc = bacc.Bacc(
        "TRN2",
        target_bir_lowering=False,
        debug=False,
        enable_asserts=True,
        num_devices=N_CORES,
    )
    d = {}
    inp = lambda name, shape, dt: d.__setitem__(
        name, nc.declare_dram_parameter(name, list(shape), dt, isOutput=False)
    )
    inp("tokT", (DF, M), BF16)
    inp("posA", (2, 5, QH), BF16)
    inp("posHL", (5, QH), BF16)
    inp("expB", (M, QH), BF16)
    inp("fmapT", (DF, QH), F32)
    inp("wq", (DF, DF), BF16)
    inp("wk", (DF, DF), BF16)
    inp("wv", (DF, DF), BF16)
    inp("w2aug", (4, 128, DF), BF16)
    inp("woutb", (DF, 1), F32)
    inp("wc5", (5, DF), BF16)
    inp("wc5s", (5, DF), BF16)
    inp("thc", (5, D_HALF), BF16)
    inp("bones", (2, 128, 8), BF16)
    inp("exp8", (2, 8, 128), BF16)
    inp("sel", (128, 2), BF16)
    inp("expP2", (2, 128), BF16)
    out = nc.declare_dram_parameter("out", [DF, QH], F32, isOutput=True)

    import os as _os

    with tile.TileContext(
        nc, trace_sim=bool(_os.environ.get("KERNEL_TRACE_SIM"))
    ) as tc:
        for r in range(reps):
            _body(nc, tc, d, out, pfx=f"r{r}_" if reps > 1 else "")
    nc.compile()
    return nc


def _brd2(ap):
    """Broadcast an AP's (p, n) view to (p, 2, n) with a stride-0 middle dim."""
    return bass.AP(ap.tensor, ap.offset, [ap.ap[0], [0, 2], ap.ap[-1]])


def _body(nc, tc, d, out_dram, pfx=""):
    mm = nc.tensor.matmul
    act = nc.scalar.activation
    dma = nc.sync.dma_start
    dmag = nc.gpsimd.dma_start

    sdt = FP8 if USE_FP8 else BF16

    with (
        tc.tile_pool(name=pfx + "const", bufs=1) as cpool,
        tc.tile_pool(name=pfx + "work", bufs=1) as wpool,
        tc.tile_pool(name=pfx + "persist", bufs=1) as ppool,
        tc.tile_pool(name=pfx + "epool", bufs=4) as epool,
        tc.tile_pool(name=pfx + "apool", bufs=4) as apool,
        tc.tile_pool(name=pfx + "psA", bufs=2, space=bass.MemorySpace.PSUM) as psA,
        tc.tile_pool(name=pfx + "psB", bufs=2, space=bass.MemorySpace.PSUM) as psB,
    ):
        # ---- load constants / inputs to SBUF ----
        def load(name, shape, dt, rearrange=None, q=None, **kw):
            t = cpool.tile(list(shape), dt, tag=name)
            src = d[name][:]
            if rearrange is not None:
                src = src.rearrange(rearrange, **kw)
            (q or dma)(t[:], src)
            return t

        fold = "(k p) d -> p k d"
        th2 = load("theta2s", (1, D_HALF), F32)
        posT2 = cpool.tile([1, 2, QH], F32, tag=pfx + "posT2")
        dma(posT2[:], d["posT"][:].rearrange("(o a) q -> o a q", o=1))
        posHL = load("posHL", (5, QH), BF16)
        wc5 = load("wc5", (5, DF), BF16)
        wc5s = load("wc5s", (5, DF), BF16)
        tokT = load("tokT", (128, 2, M), BF16, fold, p=128)
        wq = load("wq", (128, 2, DF), BF16, fold, p=128)
        wk = load("wk", (128, 2, DF), BF16, fold, p=128)
        wv = load("wv", (128, 2, DF), BF16, fold, p=128)
        bones = load("bones", (128, 2, 8), BF16, "k p h -> p k h")
        exp8 = load("exp8", (8, 2, 128), BF16, "k h d -> h k d")
        sel = load("sel", (128, 2), BF16)
        expP2 = load("expP2", (2, 128), BF16)
        w2aug = load("w2aug", (128, 4, DF), BF16, "j p d -> p j d")
        woutb = load("woutb", (128, 2, 1), F32, fold, p=128)
        fmapT = load("fmapT", (128, 2, QH), F32, fold, p=128)

        expB_sb = []
        for mc in range(8):
            bt = ppool.tile([128, QH], BF16, tag=pfx + f"expB{mc}")
            dma(bt[:], d["expB"][mc * 128 : (mc + 1) * 128, :])
            expB_sb.append(bt)

        halfpi = cpool.tile([128, 1], F32, tag=pfx + "halfpi")
        nc.vector.memset(halfpi[:], math.pi / 2.0)

        # ---- V (token-major) with ones column: vsb[mc] = (128, 8, 33) ----
        vsb = []
        for mc in range(8):
            ps = psB.tile([128, 256], F32, tag="ops")
            for kt in range(2):
                mm(
                    ps[:],
                    tokT[:, kt, mc * 128 : (mc + 1) * 128],
                    wv[:, kt, :],
                    start=(kt == 0),
                    stop=(kt == 1),
                )
            vt = ppool.tile([128, 8, 33], BF16, tag=pfx + f"v{mc}")
            nc.vector.memset(vt[:, :, 0:1], 1.0)
            nc.vector.tensor_copy(
                vt[:, :, 1:33], ps[:].rearrange("p (h e) -> p h e", h=8)
            )
            vsb.append(vt)

        # ---- Q path: coord proj (hi/lo bf16) + 2D RoPE ----
        roped = []
        for dt_i in range(2):
            r = ppool.tile([128, QH], BF16, tag=pfx + f"roped{dt_i}")
            roped.append(r)
        for dt_i in range(2):
            csl = slice(dt_i * 128, (dt_i + 1) * 128)
            for qo, qb in Q_BLOCKS:
                # angle outer product for this axis (=dt half), f32
                ang = psB.tile([128, QB], F32, tag="ops")
                mm(ang[:, :qb], th2[:, :], posT2[:, dt_i, qo : qo + qb])
                cs = wpool.tile([128, 2, QB], BF16, tag=pfx + f"cs{dt_i}")
                act(cs[:, 0, :qb], ang[:, :qb], AF.Sin, bias=halfpi[:])
                act(cs[:, 1, :qb], ang[:, :qb], AF.Sin)
                # qin (normal, swapped) for this feature half
                qin = psA.tile([128, 2, 512], F32, tag="big")
                mm(qin[:, 0, :qb], wc5[:, csl], posHL[:, qo : qo + qb])
                mm(qin[:, 1, :qb], wc5s[:, csl], posHL[:, qo : qo + qb])
                t1 = wpool.tile([128, QB], BF16, tag=pfx + "ropea")
                nc.vector.tensor_mul(t1[:, :qb], qin[:, 0, :qb], cs[:, 0, :qb])
                t2 = wpool.tile([128, QB], BF16, tag=pfx + "ropeb")
                nc.vector.tensor_mul(t2[:, :qb], qin[:, 1, :qb], cs[:, 1, :qb])
                nc.gpsimd.tensor_add(
                    roped[dt_i][:, qo : qo + qb], t1[:, :qb], t2[:, :qb]
                )

        # ---- projections + qk-norm -> fp8 DoubleRow layouts ----

        def qk_pipeline(rhs_tiles, w_sb, blocks, name, maxb):
            """Project + qk-norm.  Returns per-half fp8 DoubleRow tiles
            dr8[dt] of shape (128, 2, n): kt0 slot holds the dense normalized
            values; kt1 slot is filled by partition-shift DMAs below."""
            n = sum(b for _, b in blocks)
            dr8 = [
                ppool.tile(
                    [128, 2, n], sdt, tag=pfx + f"{name}8_{g}",
                    name=pfx + f"{name}8_{g}",
                )
                for g in range(2)
            ]
            tb = ppool.tile([128, 2, n], BF16, tag=pfx + f"{name}tb")
            for qo, qb in blocks:
                ps = psA.tile([128, 2, maxb], F32, tag="big")
                for dt_i in range(2):
                    for kt in range(2):
                        mm(
                            ps[:, dt_i, :qb],
                            w_sb[:, kt, dt_i * 128 : (dt_i + 1) * 128],
                            rhs_tiles[kt][:, qo : qo + qb],
                            start=(kt == 0),
                            stop=(kt == 1),
                        )
                nc.vector.tensor_copy(tb[:, :, qo : qo + qb], ps[:, :, :qb])
                sq = wpool.tile([128, 2, maxb], BF16, tag=pfx + f"{name}sq")
                nc.gpsimd.tensor_mul(
                    sq[:, :, :qb], tb[:, :, qo : qo + qb], tb[:, :, qo : qo + qb]
                )
                ss = psB.tile([8, maxb], F32, tag="ss")
                for dt_i in range(2):
                    mm(
                        ss[:, :qb],
                        bones[:, dt_i, :],
                        sq[:, dt_i, :qb],
                        start=(dt_i == 0),
                        stop=(dt_i == 1),
                    )
                nrm = wpool.tile([8, maxb], F32, tag=pfx + f"{name}nrm")
                act(nrm[:, :qb], ss[:, :qb], AF.Sqrt)
                inv = wpool.tile([8, maxb], BF16, tag=pfx + f"{name}inv")
                with nc.allow_low_precision(reason="bf16 inv-norm feeds bf16 matmul"):
                    nc.vector.reciprocal(inv[:, :qb], nrm[:, :qb])
                psx = psA.tile([128, 2, maxb], F32, tag="big")
                for dt_i in range(2):
                    mm(psx[:, dt_i, :qb], exp8[:, dt_i, :], inv[:, :qb])
                for dt_i in range(2):
                    nc.vector.tensor_mul(
                        dr8[dt_i][:, 0, qo : qo + qb],
                        tb[:, dt_i, qo : qo + qb],
                        psx[:, dt_i, :qb],
                    )
            # partition-shift DMAs: dk 16..31 (partitions 32h+16..32h+32 of
            # the dense kt0 slot) move to the kt1 slot of partitions 32h..+16
            if USE_FP8:
                for g in range(2):
                    for hh in range(4):
                        p0 = 32 * hh
                        dmag(
                            dr8[g][p0 : p0 + 16, 1, :],
                            dr8[g][p0 + 16 : p0 + 32, 0, :],
                        )
            return dr8

        qn8 = qk_pipeline(roped, wq, Q_BLOCKS, "qn", QB)
        tok_tiles = [tokT[:, 0, :], tokT[:, 1, :]]
        kn8 = qk_pipeline(tok_tiles, wk, K_BLOCKS, "kn", QB)

        # ---- main attention loop ----
        osb = [
            ppool.tile([128, QH], BF16, tag=pfx + f"osb{j}", name=pfx + f"osb{j}")
            for j in range(4)
        ]
        sums = wpool.tile([8, QH], BF16, tag=pfx + "sums")

        for qo, qb in Q_BLOCKS:
            for j in range(4):
                g = j // 2
                hh0 = 2 * (j % 2)
                o_ps = psB.tile([128, QB], F32, tag="ops")
                for mc in range(8):
                    s_ps = psA.tile([128, 2, 512], F32, tag="big")
                    for i, hh in enumerate((hh0, hh0 + 1)):
                        p0 = 32 * hh
                        if USE_FP8:
                            mm(
                                s_ps[:, i, :qb],
                                kn8[g][p0 : p0 + 16, :, mc * 128 : (mc + 1) * 128],
                                qn8[g][p0 : p0 + 16, :, qo : qo + qb],
                                perf_mode=DR,
                                tile_position=(p0, 0),
                            )
                        else:
                            mm(
                                s_ps[:, i, :qb],
                                kn8[g][p0 : p0 + 32, 0, mc * 128 : (mc + 1) * 128],
                                qn8[g][p0 : p0 + 32, 0, qo : qo + qb],
                                tile_position=(p0, 0),
                            )
                    e_t = epool.tile([128, 2, QB], BF16, tag="E")
                    act(e_t[:, :, :qb], s_ps[:, :, :qb], AF.Exp)
                    a_t = apool.tile([128, 2, QB], BF16, tag="A")
                    eng = nc.gpsimd if (mc >= 6 or (mc >= 4 and j < 2)) else nc.vector
                    if USE_BRD:
                        eng.tensor_mul(
                            a_t[:, :, :qb],
                            e_t[:, :, :qb],
                            _brd2(expB_sb[mc][:, qo : qo + qb]),
                        )
                    else:
                        for i in range(2):
                            eng.tensor_mul(
                                a_t[:, i, :qb],
                                e_t[:, i, :qb],
                                expB_sb[mc][:, qo : qo + qb],
                            )
                    for i, hh in enumerate((hh0, hh0 + 1)):
                        h = 4 * g + hh
                        base = 64 * (hh % 2)
                        mm(
                            o_ps[base : base + 33, :qb],
                            vsb[mc][:, h, :],
                            a_t[:, i, :qb],
                            start=(mc == 0),
                            stop=(mc == 7),
                            tile_position=(0, base),
                        )
                nc.vector.tensor_copy(osb[j][:, qo : qo + qb], o_ps[:, :qb])
                # softmax denominators (ones-column rows 0 and 64)
                dmag(
                    sums[2 * j : 2 * j + 1, qo : qo + qb],
                    osb[j][0:1, qo : qo + qb],
                )
                dmag(
                    sums[2 * j + 1 : 2 * j + 2, qo : qo + qb],
                    osb[j][64:65, qo : qo + qb],
                )

        invS = wpool.tile([8, QH], BF16, tag=pfx + "invS")
        with nc.allow_low_precision(reason="bf16 inv-denominator feeds bf16 matmul"):
            nc.vector.reciprocal(invS[:], sums[:])

        # ---- normalize + fused output projection + residual ----
        for qo, qb in Q_BLOCKS:
            for j in range(4):
                psx = psB.tile([128, QB], F32, tag="ops")
                mm(psx[:, :qb], expP[:, j, :], invS[:, qo : qo + qb])
                nc.vector.tensor_mul(
                    osb[j][:, qo : qo + qb], osb[j][:, qo : qo + qb], psx[:, :qb]
                )
            for dt_i in range(2):
                ps = psB.tile([128, QB], F32, tag="ops")
                for j in range(4):
                    mm(
                        ps[:, :qb],
                        w2aug[:, j, dt_i * 128 : (dt_i + 1) * 128],
                        osb[j][:, qo : qo + qb],
                        start=(j == 0),
                        stop=(j == 3),
                    )
                r1 = wpool.tile([128, QB], F32, tag=pfx + f"res{dt_i}")
                nc.vector.scalar_tensor_tensor(
                    r1[:, :qb],
                    ps[:, :qb],
                    woutb[:, dt_i, :],
                    fmapT[:, dt_i, qo : qo + qb],
                    op0=mybir.AluOpType.add,
                    op1=mybir.AluOpType.add,
                )
                dma(out_dram[dt_i * 128 : (dt_i + 1) * 128, qo : qo + qb], r1[:, :qb])


def kernel(
    track_tokens,
    feature_map,
    feature_positions,
    spatial_bias,
    Wq,
    Wk,
    Wv,
    Wo,
    W_out_w,
    W_out_b,
    W_coord_w,
    W_coord_b,
):
    global _NC_CACHE
    inputs = dict(
        track_tokens=track_tokens,
        feature_map=feature_map,
        feature_positions=feature_positions,
        spatial_bias=spatial_bias,
        Wq=Wq,
        Wk=Wk,
        Wv=Wv,
        Wo=Wo,
        W_out_w=W_out_w,
        W_out_b=W_out_b,
        W_coord_w=W_coord_w,
        W_coord_b=W_coord_b,
    )
    in_maps = build_in_maps(inputs)
    if _NC_CACHE is None:
        _NC_CACHE = _build_nc()
    res = run_bass_kernel_spmd(_NC_CACHE, in_maps, core_ids=list(range(N_CORES)))
    return assemble_output([res.results[c]["out"] for c in range(N_CORES)])


def build_in_maps(inputs):
    consts = _host_constants(
        np.asarray(inputs["Wq"], np.float32),
        np.asarray(inputs["Wk"], np.float32),
        np.asarray(inputs["Wv"], np.float32),
        np.asarray(inputs["Wo"], np.float32),
        np.asarray(inputs["W_out_w"], np.float32),
        np.asarray(inputs["W_out_b"], np.float32),
        np.asarray(inputs["W_coord_w"], np.float32),
        np.asarray(inputs["W_coord_b"], np.float32),
    )
    track_tokens = np.asarray(inputs["track_tokens"], np.float32)
    feature_map = np.asarray(inputs["feature_map"], np.float32)
    feature_positions = np.asarray(inputs["feature_positions"], np.float32)
    spatial_bias = np.asarray(inputs["spatial_bias"], np.float32)
    expB = np.exp(spatial_bias)

    in_maps = []
    for c in range(N_CORES):
        t, half = c // 2, c % 2
        qsl = slice(half * QH, (half + 1) * QH)
        pos = feature_positions[t, qsl].T  # (2, QH)
        hi = np.rint(pos)
        lo = pos - hi
        posHL = np.stack(
            [hi[0], lo[0], hi[1], lo[1], np.ones_like(hi[0])]
        )  # (5, QH)
        m = dict(consts)
        m["tokT"] = _bf(track_tokens[t].T)
        one = np.ones_like(pos[0])
        m["posA"] = _bf(
            np.stack(
                [
                    np.stack([hi[0], lo[0], hi[0], lo[0], one]),
                    np.stack([hi[1], lo[1], hi[1], lo[1], one]),
                ]
            )
        )
        m["posHL"] = _bf(posHL)
        m["expB"] = _bf(expB[t][:, qsl])
        m["fmapT"] = _f32(feature_map[t, qsl].T)
        in_maps.append(m)
    return in_maps


def assemble_output(per_core):
    """per_core: sequence of 8 per-core 'out' arrays, each (DF, QH)."""
    out = np.empty((T, HW, DF), np.float32)
    for c in range(N_CORES):
        t, half = c // 2, c % 2
        qsl = slice(half * QH, (half + 1) * QH)
        out[t, qsl, :] = np.asarray(per_core[c]).T
    return out
